# revision 1
# baseline (speedup 1.0000x reference)
"""Cross-attention Bass kernel for Trainium2, 8 NeuronCores, head-sharded.

Reference semantics: q = RMSNorm_head(x@Wq.T+bq), kv = c@Wkv.T+bkv (k/v
interleaved), k = RMSNorm_head(k), out = softmax(q k^T/sqrt(dh)) v, merged
heads -> [b, n, dim].

Sharding: 16 heads over 8 cores (2 heads each). Each core reads full x, c and
its weight slices; writes out[:, :, i*128:(i+1)*128] (its 2 heads are adjacent
in the output feature dim). No collectives.

Per-core pipeline (fp32 data, float32r matmuls):
  Projection phase (per 512-row chunk of x/c):
    - PE-transpose chunk -> xT/cT tiles [128k, 512seq] in SBUF
    - W-stationary projections -> qT/kT/vT [head_dims, seq] in PSUM
    - RMSNorm entirely in T layout: per-head sumsq via indicator-matmul
      (1/gamma^2 folded for k; gamma_q*gamma_k folded into Wk/bk on host),
      sqrt+reciprocal on a [2, 512] row, broadcast down partitions with a
      K=2 expander matmul, one DVE multiply.
    - V transposed to natural [m, dh] with a ones column (softmax denominator
      rides the AV matmul) and zero padding to 128 (keeps U transposable).
  Attention phase (per batch, per 512-col n-chunk, 16 m-tiles):
    S.T[m,nchunk] = kT.T @ qT (two K=64 matmuls, one per head -> row-packed
    on the PE array), exp(S.T/8) on ACT (PSUM->SBUF), U.T += V'.T @ expS.T.
    Then PE-transpose U.T -> [n,128], divide by the ones-column sum, store.

Batch 0's attention is emitted interleaved with batch 1's projections so the
ACT-heavy attention overlaps the DVE/PE-heavy projection work.
"""

import sys

sys.path.insert(0, "/opt/trn_rl_repo")

import numpy as np
from contextlib import ExitStack

import concourse.bass as bass
import concourse.tile as tile
from concourse import bacc, mybir
from concourse.bass_utils import run_bass_kernel_spmd
from concourse.masks import make_identity

F32 = mybir.dt.float32
F32R = mybir.dt.float32r

DIM = 1024
H = 16
DH = 64
B = 2
N = 2048
ROWS = B * N            # 4096 flattened rows
NC = 8
HPC = H // NC           # 2 heads per core
EPS = 1.1920928955078125e-07

NKB = DIM // 128        # 8 k-tiles
CPB = N // 512          # 4 chunks of 512 rows per batch
MT_PER_B = N // 128     # 16 m-tiles per batch

LAST_EXEC_TIME_NS = None
LAST_RESULTS = None
_LAST_IN_MAPS = None


def r(ap):
    return ap.bitcast(F32R)


class _Ctx:
    pass


def build_bass(dbg=False, reps=1):
    nc = bacc.Bacc("TRN2", target_bir_lowering=False, debug=False)
    g = _Ctx()
    g.nc = nc

    g.x = nc.dram_tensor("x", [ROWS, DIM], F32R, kind="ExternalInput")
    g.c = nc.dram_tensor("c", [ROWS, DIM], F32R, kind="ExternalInput")
    g.wq = nc.dram_tensor("wq", [DIM, 128], F32R, kind="ExternalInput")
    g.wk = nc.dram_tensor("wk", [DIM, 128], F32R, kind="ExternalInput")
    g.wv = nc.dram_tensor("wv", [DIM, 128], F32R, kind="ExternalInput")
    g.bq_d = nc.dram_tensor("bq", [128, 1], F32, kind="ExternalInput")
    g.bk_d = nc.dram_tensor("bk", [128, 1], F32, kind="ExternalInput")
    g.bv_d = nc.dram_tensor("bv", [128, 1], F32, kind="ExternalInput")
    g.gq_d = nc.dram_tensor("gq", [128, 2], F32R, kind="ExternalInput")
    g.gk_d = nc.dram_tensor("gk", [128, 2], F32R, kind="ExternalInput")
    g.out = nc.dram_tensor("out", [ROWS, 128], F32, kind="ExternalOutput")

    with tile.TileContext(nc) as tc, ExitStack() as ctx:
        g.tc = tc
        const = ctx.enter_context(tc.tile_pool(name="const", bufs=1))
        resid = ctx.enter_context(tc.tile_pool(name="resid", bufs=1))
        g.ld = ctx.enter_context(tc.tile_pool(name="ld", bufs=6))
        g.xtp = ctx.enter_context(tc.tile_pool(name="xtp", bufs=2))
        g.tmp = ctx.enter_context(tc.tile_pool(name="tmpA", bufs=2))
        g.small = ctx.enter_context(tc.tile_pool(name="small", bufs=2))
        g.esb = ctx.enter_context(tc.tile_pool(name="esb", bufs=3))
        g.usb = ctx.enter_context(tc.tile_pool(name="usb", bufs=2))
        g.osb = ctx.enter_context(tc.tile_pool(name="osb", bufs=3))
        g.rsb = ctx.enter_context(tc.tile_pool(name="rsb", bufs=4))
        # PSUM budget (8 banks): sps 2x[128,1024]=4, ups 2x[128,512]=2,
        # scr 2x[128,512]=2 shared by transposes/proj/norm/v-nat/u-transpose
        g.scr = ctx.enter_context(
            tc.tile_pool(name="scratchT", bufs=2, space="PSUM"))
        g.sps = ctx.enter_context(
            tc.tile_pool(name="sps", bufs=2, space="PSUM"))
        g.ups = ctx.enter_context(
            tc.tile_pool(name="ups", bufs=1, space="PSUM"))

        ident_f = const.tile([128, 128], F32, tag="identf")
        make_identity(nc, ident_f[:])
        g.ident = const.tile([128, 128], F32R, tag="ident")
        nc.vector.tensor_copy(g.ident[:], ident_f[:])

        g.wq_sb = const.tile([128, NKB, 128], F32R, tag="wq")
        g.wk_sb = const.tile([128, NKB, 128], F32R, tag="wk")
        g.wv_sb = const.tile([128, NKB, 128], F32R, tag="wv")
        for kb in range(NKB):
            nc.sync.dma_start(g.wq_sb[:, kb], g.wq[kb * 128:(kb + 1) * 128, :])
            nc.sync.dma_start(g.wk_sb[:, kb], g.wk[kb * 128:(kb + 1) * 128, :])
            nc.sync.dma_start(g.wv_sb[:, kb], g.wv[kb * 128:(kb + 1) * 128, :])
        g.bq_sb = const.tile([128, 1], F32, tag="bq")
        g.bk_sb = const.tile([128, 1], F32, tag="bk")
        g.bv_sb = const.tile([128, 1], F32, tag="bv")
        g.gq_sb = const.tile([128, 2], F32R, tag="gq")
        g.gk_sb = const.tile([128, 2], F32R, tag="gk")
        nc.sync.dma_start(g.bq_sb[:], g.bq_d[:])
        nc.sync.dma_start(g.bk_sb[:], g.bk_d[:])
        nc.sync.dma_start(g.bv_sb[:], g.bv_d[:])
        nc.sync.dma_start(g.gq_sb[:], g.gq_d[:])
        nc.sync.dma_start(g.gk_sb[:], g.gk_d[:])
        g.eps_sb = const.tile([128, 1], F32, tag="eps")
        nc.gpsimd.memset(g.eps_sb[:], EPS)

        # expander: expand[x, y] = 1 iff y//64 == x  (rb[p] = rinv[p//64])
        expand_f = const.tile([2, 128], F32, tag="expand_f")
        nc.gpsimd.memset(expand_f[:], 0.0)
        nc.gpsimd.affine_select(
            out=expand_f[:], in_=expand_f[:],
            compare_op=mybir.AluOpType.is_ge, fill=1.0,
            base=-64, pattern=[[1, 128]], channel_multiplier=-64)
        nc.gpsimd.affine_select(
            out=expand_f[:], in_=expand_f[:],
            compare_op=mybir.AluOpType.is_ge, fill=0.0,
            base=0, pattern=[[1, 128]], channel_multiplier=-64)
        g.expand_r = const.tile([2, 128], F32R, tag="expand_r")
        nc.vector.tensor_copy(g.expand_r[:], expand_f[:])

        ones64 = const.tile([128, 64], F32, tag="ones64")
        nc.gpsimd.memset(ones64[:], 1.0)
        z1 = const.tile([128, 1], F32, tag="z1")
        nc.gpsimd.memset(z1[:], 0.0)

        # per-batch residents
        g.qt = [resid.tile([128, N], F32R, tag=f"qt{b}", name=f"qt{b}")
                for b in range(B)]
        g.kt = [resid.tile([128, N], F32R, tag=f"kt{b}", name=f"kt{b}")
                for b in range(B)]
        g.v2 = [resid.tile([128, MT_PER_B, 2, 128], F32R, tag=f"v2{b}",
                           name=f"v2{b}")
                for b in range(B)]
        for b in range(B):
            nc.vector.tensor_copy(
                g.v2[b][:, :, :, 64:65],
                ones64[:, 0:MT_PER_B * 2].rearrange(
                    "p (a b c) -> p a b c", a=MT_PER_B, b=2))
            nc.vector.tensor_copy(
                g.v2[b][:, :, :, 65:128],
                z1[:].broadcast_to((128, MT_PER_B, 2, 63)))

        for _ in range(reps):
            # batch 0 projections, then batch-0 attention interleaved with
            # batch-1 projections, then batch-1 attention
            for ch in range(CPB):
                _proj_chunk(g, 0, ch)
            for ch in range(CPB):
                _attn_chunk(g, 0, ch)
                _proj_chunk(g, 1, ch)
            for ch in range(CPB):
                _attn_chunk(g, 1, ch)

        if dbg:
            qt_d = nc.dram_tensor("qt_dbg", [128, ROWS], F32,
                                  kind="ExternalOutput")
            kt_d = nc.dram_tensor("kt_dbg", [128, ROWS], F32,
                                  kind="ExternalOutput")
            v2_d = nc.dram_tensor("v2_dbg", [128, ROWS * 2], F32,
                                  kind="ExternalOutput")
            for b in range(B):
                nc.sync.dma_start(qt_d[:, b * N:(b + 1) * N],
                                  g.qt[b][:].bitcast(F32))
                nc.sync.dma_start(kt_d[:, b * N:(b + 1) * N],
                                  g.kt[b][:].bitcast(F32))
                nc.sync.dma_start(
                    v2_d[:, b * N * 2:(b + 1) * N * 2],
                    g.v2[b][:].bitcast(F32).rearrange("p a b e -> p (a b e)"))

    nc.compile()
    return nc


def _transpose_chunk(g, src, row0):
    """Load 4 [128,1024] tiles at row0, PE-transpose to 8 [128k, 512] tiles."""
    nc = g.nc
    tiles = []
    for t in range(4):
        lt = g.ld.tile([128, DIM], F32R, tag="ld")
        nc.sync.dma_start(lt[:], src[row0 + t * 128: row0 + (t + 1) * 128, :])
        tiles.append(lt)
    outs = []
    for kb in range(NKB):
        ps = g.scr.tile([128, 512], F32, tag="scr")
        for t in range(4):
            nc.tensor.transpose(
                r(ps[:, t * 128:(t + 1) * 128]),
                tiles[t][:, kb * 128:(kb + 1) * 128],
                g.ident[:])
        sb = g.xtp.tile([128, 512], F32R, tag=f"xt{kb}")
        nc.vector.tensor_copy(sb[:], ps[:])
        outs.append(sb)
    return outs


def _norm_T(g, lin_ps, bias_sb, g_sb, dst_ap):
    """RMSNorm in T layout: dst = (lin+bias) * rsqrt(mean(sq)+eps) per head."""
    nc = g.nc
    s_sb = g.tmp.tile([128, 512], F32, tag="lin")
    nc.vector.tensor_scalar_add(s_sb[:], lin_ps[:], bias_sb[:])
    sq = g.tmp.tile([128, 512], F32R, tag="sq")
    nc.vector.tensor_tensor(
        out=sq[:], in0=s_sb[:], in1=s_sb[:], op=mybir.AluOpType.mult)
    ss = g.scr.tile([2, 512], F32, tag="scr", name="ss")
    nc.tensor.matmul(ss[:], g_sb[:], sq[:])
    rms = g.small.tile([2, 512], F32, tag="rms")
    nc.scalar.activation(
        rms[:], ss[:], mybir.ActivationFunctionType.Sqrt,
        bias=g.eps_sb[0:2, :], scale=1.0 / DH)
    rinv = g.small.tile([2, 512], F32R, tag="rinv")
    with nc.allow_low_precision(reason="f32r is fp32-width"):
        nc.vector.reciprocal(rinv[:], rms[:])
    rb = g.scr.tile([128, 512], F32, tag="scr", name="rb")
    nc.tensor.matmul(rb[:], g.expand_r[:], rinv[:])
    nc.vector.tensor_tensor(
        out=dst_ap, in0=s_sb[:], in1=rb[:], op=mybir.AluOpType.mult)


def _proj_chunk(g, b, ch):
    nc = g.nc
    row0 = b * N + ch * 512
    cols = bass.ds(ch * 512, 512)

    xt = _transpose_chunk(g, g.x, row0)
    q_ps = g.scr.tile([128, 512], F32, tag="scr")
    for kb in range(NKB):
        nc.tensor.matmul(q_ps[:], g.wq_sb[:, kb], xt[kb][:],
                         start=(kb == 0), stop=(kb == NKB - 1))
    _norm_T(g, q_ps, g.bq_sb, g.gq_sb, g.qt[b][:, cols])

    ct = _transpose_chunk(g, g.c, row0)
    k_ps = g.scr.tile([128, 512], F32, tag="scr")
    for kb in range(NKB):
        nc.tensor.matmul(k_ps[:], g.wk_sb[:, kb], ct[kb][:],
                         start=(kb == 0), stop=(kb == NKB - 1))
    _norm_T(g, k_ps, g.bk_sb, g.gk_sb, g.kt[b][:, cols])

    v_ps = g.scr.tile([128, 512], F32, tag="scr")
    for kb in range(NKB):
        nc.tensor.matmul(v_ps[:], g.wv_sb[:, kb], ct[kb][:],
                         start=(kb == 0), stop=(kb == NKB - 1))
    v_sb = g.tmp.tile([128, 512], F32R, tag="vsb")
    nc.vector.tensor_scalar_add(v_sb[:], v_ps[:], g.bv_sb[:])
    vn = g.scr.tile([128, 512], F32, tag="scr")
    for t in range(4):
        nc.tensor.transpose(
            r(vn[:, t * 128:(t + 1) * 128]),
            v_sb[:, t * 128:(t + 1) * 128],
            g.ident[:])
    mt0 = ch * 4
    nc.vector.tensor_copy(
        g.v2[b][:, mt0:mt0 + 4, :, 0:64],
        vn[:].rearrange("p (t h e) -> p t h e", t=4, h=2))


def _attn_chunk(g, b, ch):
    nc = g.nc
    n0 = b * N + ch * 512
    ncols = bass.ds(ch * 512, 512)
    qt, kt, v2 = g.qt[b], g.kt[b], g.v2[b]
    uA = g.ups.tile([128, 512], F32, tag="uA")
    uB = g.ups.tile([128, 512], F32, tag="uB")
    for mt in range(MT_PER_B):
        mcols = bass.ds(mt * 128, 128)
        s_ps = g.sps.tile([128, 1024], F32, tag="s")
        nc.tensor.matmul(s_ps[:, 0:512], kt[0:64, mcols], qt[0:64, ncols])
        nc.tensor.matmul(s_ps[:, 512:1024], kt[64:128, mcols],
                         qt[64:128, ncols])
        e_sb = g.esb.tile([128, 1024], F32R, tag="e")
        nc.scalar.activation(
            e_sb[:], s_ps[:], mybir.ActivationFunctionType.Exp, scale=0.125)
        nc.tensor.matmul(uA[:], v2[:, mt, 0], e_sb[:, 0:512],
                         start=(mt == 0), stop=(mt == MT_PER_B - 1),
                         skip_group_check=True)
        nc.tensor.matmul(uB[:], v2[:, mt, 1], e_sb[:, 512:1024],
                         start=(mt == 0), stop=(mt == MT_PER_B - 1),
                         skip_group_check=True)
    uA_sb = g.usb.tile([128, 512], F32R, tag="uAs")
    uB_sb = g.usb.tile([128, 512], F32R, tag="uBs")
    nc.vector.tensor_copy(uA_sb[:], uA[:])
    nc.vector.tensor_copy(uB_sb[:], uB[:])
    for nt in range(4):
        t_ps = g.scr.tile([128, 256], F32, tag="scr")
        nc.tensor.transpose(
            r(t_ps[:, 0:128]), uA_sb[:, nt * 128:(nt + 1) * 128], g.ident[:])
        nc.tensor.transpose(
            r(t_ps[:, 128:256]), uB_sb[:, nt * 128:(nt + 1) * 128],
            g.ident[:])
        rA = g.rsb.tile([128, 1], F32, tag="rA")
        rB = g.rsb.tile([128, 1], F32, tag="rB")
        nc.vector.reciprocal(rA[:], t_ps[:, 64:65])
        nc.vector.reciprocal(rB[:], t_ps[:, 192:193])
        o_sb = g.osb.tile([128, 128], F32, tag="o")
        nc.vector.tensor_scalar_mul(o_sb[:, 0:64], t_ps[:, 0:64], rA[:])
        nc.vector.tensor_scalar_mul(o_sb[:, 64:128], t_ps[:, 128:192], rB[:])
        nc.sync.dma_start(
            g.out[n0 + nt * 128: n0 + (nt + 1) * 128, :], o_sb[:])


_CACHED_NC = None


def kernel(x, c, Wq, bq, Wkv, bkv, q_gamma, k_gamma, _trace=False, _dbg=False):
    global LAST_EXEC_TIME_NS, LAST_RESULTS, _CACHED_NC, _LAST_IN_MAPS

    x = np.asarray(x, dtype=np.float32)
    c = np.asarray(c, dtype=np.float32)
    Wq = np.asarray(Wq, dtype=np.float32)
    bq = np.asarray(bq, dtype=np.float32)
    Wkv = np.asarray(Wkv, dtype=np.float32)
    bkv = np.asarray(bkv, dtype=np.float32)
    q_gamma = np.asarray(q_gamma, dtype=np.float32)
    k_gamma = np.asarray(k_gamma, dtype=np.float32)

    b, n, _ = x.shape
    x_flat = np.ascontiguousarray(x.reshape(ROWS, DIM))
    c_flat = np.ascontiguousarray(c.reshape(ROWS, DIM))

    g2 = q_gamma * k_gamma                      # [64]
    g2_2 = np.tile(g2, HPC)                     # [128]
    d2 = np.arange(DH)

    in_maps = []
    for i in range(NC):
        h0 = i * HPC
        rows_q = np.concatenate(
            [h * DH + d2 for h in range(h0, h0 + HPC)])
        k_rows = np.concatenate(
            [h * 2 * DH + 2 * d2 for h in range(h0, h0 + HPC)])
        v_rows = k_rows + 1

        wq_t = np.ascontiguousarray(Wq[rows_q].T)
        wk_t = np.ascontiguousarray((Wkv[k_rows] * g2_2[:, None]).T)
        wv_t = np.ascontiguousarray(Wkv[v_rows].T)
        bq_l = np.ascontiguousarray(bq[rows_q].reshape(128, 1))
        bk_l = np.ascontiguousarray((bkv[k_rows] * g2_2).reshape(128, 1))
        bv_l = np.ascontiguousarray(bkv[v_rows].reshape(128, 1))

        gq_l = np.zeros((128, 2), dtype=np.float32)
        gk_l = np.zeros((128, 2), dtype=np.float32)
        for h in range(HPC):
            gq_l[h * DH:(h + 1) * DH, h] = 1.0
            gk_l[h * DH:(h + 1) * DH, h] = 1.0 / (g2 * g2)
        in_maps.append({
            "x": x_flat, "c": c_flat,
            "wq": wq_t, "wk": wk_t, "wv": wv_t,
            "bq": bq_l, "bk": bk_l, "bv": bv_l,
            "gq": gq_l, "gk": gk_l,
        })

    _LAST_IN_MAPS = in_maps
    if _CACHED_NC is None:
        _CACHED_NC = build_bass(dbg=_dbg)
    nc = _CACHED_NC

    res = run_bass_kernel_spmd(
        nc, in_maps, core_ids=list(range(NC)), trace=_trace)
    LAST_EXEC_TIME_NS = res.exec_time_ns
    LAST_RESULTS = res

    outs = [res.results[i]["out"] for i in range(NC)]
    full = np.concatenate(outs, axis=1)
    return full.reshape(b, n, DIM)

